# revision 2
# baseline (speedup 1.0000x reference)
"""Trainium2 Bass kernel for the AllPairs triplet-index sampling problem.

Problem (from the reference):
  B=1024 embeddings with balanced labels (C=128 classes, S=8 per class).
  Output is the triplet index expansion
    anchor_idx = repeat(pa, NNEG), pos_idx = repeat(pp, NNEG),
    neg_idx    = neg_per_anchor[pa].reshape(-1)
  where (pa, pp) enumerates the NPOS=B*(S-1)=7168 positive pairs in
  row-major order and neg_per_anchor[i] lists the NNEG=1016 ascending
  indices j with labels[j] != labels[i].

Sharding: the positive-pair axis is split into 8 contiguous slabs of 896
pairs = 128 anchors per core (pair k belongs to anchor k//7, so a
contiguous pair slab is a contiguous anchor slab). Each core computes,
for its 128 anchors (one anchor per SBUF partition):
  - the 8 ascending same-class member indices (via a prefix-sum rank and
    a gpsimd local_scatter compaction),
  - the 7 positive indices pp (members minus self),
  - the 1016 ascending negative indices (second local_scatter),
then materializes small per-row tiles and fans them out to the
[128, 7*1016] output slabs with broadcast-AP DMAs (the kernel is HBM
write bound: 3 x 3.64 MB per core).

Outputs are exact int32 indices; everything on-device is computed in
f32/int16 (all values < 2^11, exactly representable).
"""

import numpy as np

from concourse import bacc, mybir, tile
from concourse.bass_utils import run_bass_kernel_spmd

B = 1024          # batch
C = 128           # classes
S = B // C        # samples per class (8)
PER = S - 1       # positives per anchor (7)
NNEG = B - S      # negatives per anchor (1016)
ACH = 128         # anchors per core
N_CORES = 8
REP = 254         # pos_rep inner width; NNEG = 4 * REP
BIG = 2048.0

f32 = mybir.dt.float32
i32 = mybir.dt.int32
i16 = mybir.dt.int16

_NC = None


def _build():
    global _NC
    if _NC is not None:
        return _NC
    nc = bacc.Bacc("TRN2", target_bir_lowering=False, debug=False,
                   num_devices=N_CORES)

    lab_rep = nc.declare_dram_parameter("lab_rep", [ACH, B], f32, isOutput=False)
    iota_f = nc.declare_dram_parameter("iota_f", [ACH, B], f32, isOutput=False)
    iota16 = nc.declare_dram_parameter("iota16", [ACH, B], i16, isOutput=False)
    lab_anchor = nc.declare_dram_parameter("lab_anchor", [ACH, 1], f32, isOutput=False)
    anchor_f = nc.declare_dram_parameter("anchor_f", [ACH, 1], f32, isOutput=False)

    anchor_out = nc.declare_dram_parameter("anchor_out", [ACH, PER, NNEG], i32, isOutput=True)
    pos_out = nc.declare_dram_parameter("pos_out", [ACH, PER, NNEG // REP, REP], i32, isOutput=True)
    neg_out = nc.declare_dram_parameter("neg_out", [ACH, PER, NNEG], i32, isOutput=True)

    op = mybir.AluOpType
    with tile.TileContext(nc) as tc:
        with tc.tile_pool(name="p", bufs=1) as pool:
            t_lab = pool.tile([ACH, B], f32)
            t_iota = pool.tile([ACH, B], f32)
            t_iota16 = pool.tile([ACH, B], i16)
            t_laba = pool.tile([ACH, 1], f32)
            t_anc = pool.tile([ACH, 1], f32)
            t_ones = pool.tile([ACH, B], f32)
            t_eq = pool.tile([ACH, B], f32)
            t_rank = pool.tile([ACH, B], f32)
            t_tmpa = pool.tile([ACH, B], f32)
            t_idxq = pool.tile([ACH, B], i16)
            t_tmpb = pool.tile([ACH, B], f32)
            t_idxn = pool.tile([ACH, B], i16)
            t_qbuf = pool.tile([ACH, 16], i16)
            t_qf = pool.tile([ACH, 16], f32)
            t_negbuf = pool.tile([ACH, B], i16)
            t_neg32 = pool.tile([ACH, NNEG], i32)
            t_anc32 = pool.tile([ACH, NNEG], i32)
            t_cm = pool.tile([ACH, PER], f32)
            t_dq = pool.tile([ACH, PER], f32)
            t_dq2 = pool.tile([ACH, PER], f32)
            t_ppf = pool.tile([ACH, PER], f32)
            t_pos32 = pool.tile([ACH, PER, REP], i32)

            # inputs
            nc.sync.dma_start(t_lab[:, :], lab_rep[:, :])
            nc.sync.dma_start(t_iota[:, :], iota_f[:, :])
            nc.sync.dma_start(t_iota16[:, :], iota16[:, :])
            nc.sync.dma_start(t_laba[:, :], lab_anchor[:, :])
            nc.sync.dma_start(t_anc[:, :], anchor_f[:, :])

            nc.vector.memset(t_ones[:, :], 1.0)

            # anchor slab: every element of row p is the global anchor id.
            nc.vector.tensor_scalar(t_anc32[:, :], t_ones[:, :NNEG],
                                    0.0, t_anc[:, 0:1], op.mult, op.add)
            nc.sync.dma_start(
                anchor_out[:, :, :],
                t_anc32[:, :].unsqueeze(1).broadcast_to([ACH, PER, NNEG]))

            # eq[p, j] = labels[j] == labels[anchor_p]
            nc.vector.tensor_scalar(t_eq[:, :], t_lab[:, :],
                                    t_laba[:, 0:1], None, op.is_equal)
            # rank[p, j] = inclusive running count of members
            nc.vector.tensor_tensor_scan(t_rank[:, :], t_ones[:, :], t_eq[:, :],
                                         0.0, op.mult, op.add)
            # member slot = rank-1 in [0,8); non-members pushed negative
            nc.vector.tensor_scalar(t_tmpa[:, :], t_eq[:, :],
                                    BIG, -(BIG + 1.0), op.mult, op.add)
            nc.vector.tensor_tensor(t_idxq[:, :], t_rank[:, :], t_tmpa[:, :], op.add)
            # negative-rank slot = j - rank for non-members; members negative
            nc.vector.scalar_tensor_tensor(t_tmpb[:, :], t_rank[:, :], -1.0,
                                           t_iota[:, :], op.mult, op.add)
            nc.vector.scalar_tensor_tensor(t_idxn[:, :], t_eq[:, :], -BIG,
                                           t_tmpb[:, :], op.mult, op.add)

            # compaction: q = ascending member indices, negbuf = ascending negatives
            nc.gpsimd.local_scatter(t_qbuf[:, :], t_iota16[:, :], t_idxq[:, :],
                                    channels=ACH, num_elems=16, num_idxs=B)
            nc.gpsimd.local_scatter(t_negbuf[:, :], t_iota16[:, :], t_idxn[:, :],
                                    channels=ACH, num_elems=B, num_idxs=B)

            nc.vector.tensor_copy(t_neg32[:, :], t_negbuf[:, :NNEG])
            nc.sync.dma_start(
                neg_out[:, :, :],
                t_neg32[:, :].unsqueeze(1).broadcast_to([ACH, PER, NNEG]))

            # pp_t = q_t if q_t < anchor else q_{t+1}
            nc.vector.tensor_copy(t_qf[:, :], t_qbuf[:, :])
            nc.vector.tensor_scalar(t_cm[:, :], t_qf[:, 0:PER],
                                    t_anc[:, 0:1], None, op.is_lt)
            nc.vector.tensor_tensor(t_dq[:, :], t_qf[:, 0:PER], t_qf[:, 1:S], op.subtract)
            nc.vector.tensor_tensor(t_dq2[:, :], t_cm[:, :], t_dq[:, :], op.mult)
            nc.vector.tensor_tensor(t_ppf[:, :], t_qf[:, 1:S], t_dq2[:, :], op.add)
            for t in range(PER):
                nc.vector.tensor_scalar(t_pos32[:, t, :], t_ones[:, :REP],
                                        0.0, t_ppf[:, t:t + 1], op.mult, op.add)
            for r in range(NNEG // REP):
                nc.sync.dma_start(pos_out[:, :, r, :], t_pos32[:, :, :])
    nc.compile()
    _NC = nc
    return nc


def _in_maps(labels):
    lab_f = np.asarray(labels).astype(np.float32)
    lab_rep = np.ascontiguousarray(np.broadcast_to(lab_f[None, :], (ACH, B)))
    iota_f = np.ascontiguousarray(
        np.broadcast_to(np.arange(B, dtype=np.float32)[None, :], (ACH, B)))
    iota16 = np.ascontiguousarray(
        np.broadcast_to(np.arange(B, dtype=np.int16)[None, :], (ACH, B)))
    maps = []
    for d in range(N_CORES):
        sl = slice(d * ACH, (d + 1) * ACH)
        maps.append({
            "lab_rep": lab_rep,
            "iota_f": iota_f,
            "iota16": iota16,
            "lab_anchor": lab_f[sl].reshape(ACH, 1).copy(),
            "anchor_f": np.arange(d * ACH, (d + 1) * ACH,
                                  dtype=np.float32).reshape(ACH, 1),
        })
    return maps


def _gather(results):
    anchor = np.concatenate([results[d]["anchor_out"].reshape(-1)
                             for d in range(N_CORES)]).astype(np.int32, copy=False)
    pos = np.concatenate([results[d]["pos_out"].reshape(-1)
                          for d in range(N_CORES)]).astype(np.int32, copy=False)
    neg = np.concatenate([results[d]["neg_out"].reshape(-1)
                          for d in range(N_CORES)]).astype(np.int32, copy=False)
    return anchor, pos, neg


def run(labels, trace=False):
    nc = _build()
    res = run_bass_kernel_spmd(nc, _in_maps(labels),
                               core_ids=list(range(N_CORES)), trace=trace)
    return _gather(res.results), res


def kernel(embeddings=None, labels=None, **_):
    (anchor, pos, neg), _res = run(labels, trace=False)
    return anchor, pos, neg


# revision 3
# speedup vs baseline: 1.0610x; 1.0610x over previous
"""Trainium2 Bass kernel for the AllPairs triplet-index sampling problem.

Problem (from the reference):
  B=1024 embeddings with balanced labels (C=128 classes, S=8 per class).
  Output is the triplet index expansion
    anchor_idx = repeat(pa, NNEG), pos_idx = repeat(pp, NNEG),
    neg_idx    = neg_per_anchor[pa].reshape(-1)
  where (pa, pp) enumerates the NPOS=B*(S-1)=7168 positive pairs in
  row-major order and neg_per_anchor[i] lists the NNEG=1016 ascending
  indices j with labels[j] != labels[i].

Sharding: the positive-pair axis is split into 8 contiguous slabs of 896
pairs = 128 anchors per core (pair k belongs to anchor k//7, so a
contiguous pair slab is a contiguous anchor slab). Each core computes,
for its 128 anchors (one anchor per SBUF partition), via a prefix-sum
rank and ONE gpsimd local_scatter (a bijection on [0,1024): non-members
land at their negative-rank 0..1015, members at 1016..1023 in member
order), then fans the small per-row tiles out to the [128, 7*1016]
output slabs (HBM write bound: 3 x 3.64 MB per core).

Outputs are exact int32 indices; on-device compute is f32/int16 (all
values < 2^11, exactly representable).
"""

import numpy as np

from concourse import bacc, mybir, tile
from concourse.bass_utils import run_bass_kernel_spmd

B = 1024          # batch
C = 128           # classes
S = B // C        # samples per class (8)
PER = S - 1       # positives per anchor (7)
NNEG = B - S      # negatives per anchor (1016)
ACH = 128         # anchors per core
N_CORES = 8
WID = PER * NNEG  # 7112 output row width

f32 = mybir.dt.float32
i32 = mybir.dt.int32
i16 = mybir.dt.int16

_NC = None


def _build():
    global _NC
    if _NC is not None:
        return _NC
    nc = bacc.Bacc("TRN2", target_bir_lowering=False, debug=False,
                   num_devices=N_CORES)

    # packed int16 input: [:, 0:B] = labels (replicated), [:, B:2B] = iota j
    pack16 = nc.declare_dram_parameter("pack16", [ACH, 2 * B], i16, isOutput=False)
    # tiny per-core f32 input: [:, 0] = labels[anchor_p], [:, 1] = anchor id
    tiny = nc.declare_dram_parameter("tiny", [ACH, 2], f32, isOutput=False)

    anchor_out = nc.declare_dram_parameter("anchor_out", [ACH, PER, NNEG], i32, isOutput=True)
    pos_out = nc.declare_dram_parameter("pos_out", [ACH, PER, NNEG], i32, isOutput=True)
    neg_out = nc.declare_dram_parameter("neg_out", [ACH, PER, NNEG], i32, isOutput=True)

    op = mybir.AluOpType
    with tile.TileContext(nc) as tc:
        with tc.tile_pool(name="p", bufs=1) as pool:
            t_pack = pool.tile([ACH, 2 * B], i16)
            t_tiny = pool.tile([ACH, 2], f32)
            t_ones = pool.tile([ACH, B], f32)
            t_iota = pool.tile([ACH, B], f32)
            t_eq = pool.tile([ACH, B], f32)
            t_rank = pool.tile([ACH, B], f32)
            t_tmpb = pool.tile([ACH, B], f32)   # j - rank
            t_s2 = pool.tile([ACH, B], f32)     # 2*rank - j
            t_u = pool.tile([ACH, B], f32)      # tmpb + 1015*eq
            t_w = pool.tile([ACH, B], f32)      # eq * s2
            t_idx = pool.tile([ACH, B], i16)
            t_scat = pool.tile([ACH, B], i16)
            t_neg32 = pool.tile([ACH, NNEG], i32)
            t_anc32 = pool.tile([ACH, NNEG], i32)
            t_qf = pool.tile([ACH, S], f32)
            t_cm = pool.tile([ACH, PER], f32)
            t_dq = pool.tile([ACH, PER], f32)
            t_dq2 = pool.tile([ACH, PER], f32)
            t_ppf = pool.tile([ACH, PER], f32)
            t_pos32 = pool.tile([ACH, PER, NNEG], i32)

            lab16 = t_pack[:, 0:B]
            iota16 = t_pack[:, B:2 * B]

            # inputs: tiny first (anchor path depends only on it)
            nc.sync.dma_start(t_tiny[:, :], tiny[:, :])
            nc.sync.dma_start(t_pack[:, :], pack16[:, :])

            nc.vector.memset(t_ones[:, :], 1.0)

            # anchor slab: every element of row p is the global anchor id;
            # fanned out x7 by a broadcast-AP DMA on the ACT HWDGE ring.
            nc.vector.tensor_scalar(t_anc32[:, :], t_ones[:, :NNEG],
                                    0.0, t_tiny[:, 1:2], op.mult, op.add)
            nc.scalar.dma_start(
                anchor_out[:, :, :],
                t_anc32[:, :].unsqueeze(1).broadcast_to([ACH, PER, NNEG]))

            # iota as f32 (off the critical path, ACT engine)
            nc.scalar.copy(t_iota[:, :], iota16)

            # eq[p, j] = labels[j] == labels[anchor_p]
            nc.vector.tensor_scalar(t_eq[:, :], lab16,
                                    t_tiny[:, 0:1], None, op.is_equal)
            # rank[p, j] = inclusive running count of members
            nc.vector.tensor_tensor_scan(t_rank[:, :], t_ones[:, :], t_eq[:, :],
                                         0.0, op.mult, op.add)
            # bijective scatter index:
            #   non-member j -> j - rank           (0..1015, ascending negatives)
            #   member j     -> 1015 + rank        (1016..1023, ascending members)
            # idx = (j - rank) + eq*(1015 + 2*rank - j)
            nc.vector.scalar_tensor_tensor(t_tmpb[:, :], t_rank[:, :], -1.0,
                                           t_iota[:, :], op.mult, op.add)
            nc.vector.scalar_tensor_tensor(t_s2[:, :], t_rank[:, :], 2.0,
                                           t_iota[:, :], op.mult, op.subtract)
            nc.vector.scalar_tensor_tensor(t_u[:, :], t_eq[:, :], 1015.0,
                                           t_tmpb[:, :], op.mult, op.add)
            nc.vector.tensor_tensor(t_w[:, :], t_eq[:, :], t_s2[:, :], op.mult)
            nc.vector.tensor_tensor(t_idx[:, :], t_u[:, :], t_w[:, :], op.add)

            nc.gpsimd.local_scatter(t_scat[:, :], iota16, t_idx[:, :],
                                    channels=ACH, num_elems=B, num_idxs=B)

            # negatives: first 1016 slots, fanned out x7 on the sync ring
            nc.vector.tensor_copy(t_neg32[:, :], t_scat[:, :NNEG])
            nc.sync.dma_start(
                neg_out[:, :, :],
                t_neg32[:, :].unsqueeze(1).broadcast_to([ACH, PER, NNEG]))

            # members q = slots 1016..1023; pp_t = q_t if q_t < anchor else q_{t+1}
            nc.vector.tensor_copy(t_qf[:, :], t_scat[:, NNEG:B])
            nc.vector.tensor_scalar(t_cm[:, :], t_qf[:, 0:PER],
                                    t_tiny[:, 1:2], None, op.is_lt)
            nc.vector.tensor_tensor(t_dq[:, :], t_qf[:, 0:PER], t_qf[:, 1:S], op.subtract)
            nc.vector.tensor_tensor(t_dq2[:, :], t_cm[:, :], t_dq[:, :], op.mult)
            nc.vector.tensor_tensor(t_ppf[:, :], t_qf[:, 1:S], t_dq2[:, :], op.add)
            for t in range(PER):
                nc.vector.tensor_scalar(t_pos32[:, t, :], t_ones[:, :NNEG],
                                        0.0, t_ppf[:, t:t + 1], op.mult, op.add)
            nc.scalar.dma_start(pos_out[:, :, :], t_pos32[:, :, :])
    nc.compile()
    _NC = nc
    return nc


def _in_maps(labels):
    lab = np.asarray(labels).astype(np.int16)
    pack = np.empty((ACH, 2 * B), dtype=np.int16)
    pack[:, 0:B] = lab[None, :]
    pack[:, B:2 * B] = np.arange(B, dtype=np.int16)[None, :]
    maps = []
    for d in range(N_CORES):
        sl = slice(d * ACH, (d + 1) * ACH)
        tiny = np.empty((ACH, 2), dtype=np.float32)
        tiny[:, 0] = lab[sl].astype(np.float32)
        tiny[:, 1] = np.arange(d * ACH, (d + 1) * ACH, dtype=np.float32)
        maps.append({"pack16": pack, "tiny": tiny})
    return maps


def _gather(results):
    anchor = np.concatenate([results[d]["anchor_out"].reshape(-1)
                             for d in range(N_CORES)]).astype(np.int32, copy=False)
    pos = np.concatenate([results[d]["pos_out"].reshape(-1)
                          for d in range(N_CORES)]).astype(np.int32, copy=False)
    neg = np.concatenate([results[d]["neg_out"].reshape(-1)
                          for d in range(N_CORES)]).astype(np.int32, copy=False)
    return anchor, pos, neg


def run(labels, trace=False):
    nc = _build()
    res = run_bass_kernel_spmd(nc, _in_maps(labels),
                               core_ids=list(range(N_CORES)), trace=trace)
    return _gather(res.results), res


def kernel(embeddings=None, labels=None, **_):
    (anchor, pos, neg), _res = run(labels, trace=False)
    return anchor, pos, neg


# revision 6
# speedup vs baseline: 1.2205x; 1.1503x over previous
"""Trainium2 Bass kernel for the AllPairs triplet-index sampling problem.

Problem (from the reference):
  B=1024 embeddings with balanced labels (C=128 classes, S=8 per class).
  Output is the triplet index expansion
    anchor_idx = repeat(pa, NNEG), pos_idx = repeat(pp, NNEG),
    neg_idx    = neg_per_anchor[pa].reshape(-1)
  where (pa, pp) enumerates the NPOS=B*(S-1)=7168 positive pairs in
  row-major order and neg_per_anchor[i] lists the NNEG=1016 ascending
  indices j with labels[j] != labels[i].

Sharding: the positive-pair axis is split into 8 contiguous slabs of 896
pairs = 128 anchors per core (pair k belongs to anchor k//7, so a
contiguous pair slab is a contiguous anchor slab). Each core handles its
128 anchors as the 128 SBUF partitions.

Per-core algorithm (one anchor per partition, int16 compute for the DVE
2x perf mode; every value < 2^11 so int16/f32 are exact):
  eq[p,j]   = labels[j] == labels[anchor_p]
  rank[p,j] = prefix sum of eq (tensor_tensor_scan)
  idx[p,j]  = j - rank + eq*(1024 - j)   -- a bijection on [0,1024):
              non-members land at their negative-rank 0..1015 ascending,
              members at 1024-rank (1016..1023, descending member order)
  scat      = one gpsimd local_scatter of j by idx
  negatives = scat[:, 0:1016], members u = scat[:, 1016:1024]
  pp        = the 7 members != anchor, via a vectorized select on u
The three [128, 7*1016] output slabs are then written HBM-roofline
style: anchor (per-partition constant) on the ACT HWDGE ring, negatives
and positives via SWDGE DMAs that cast int16->int32 inline, with x7
broadcast access patterns so SBUF holds only one copy.
"""

import numpy as np

from concourse import bacc, mybir, tile
from concourse.bass_utils import run_bass_kernel_spmd

B = 1024          # batch
C = 128           # classes
S = B // C        # samples per class (8)
PER = S - 1       # positives per anchor (7)
NNEG = B - S      # negatives per anchor (1016)
ACH = 128         # anchors per core
N_CORES = 8

f32 = mybir.dt.float32
i32 = mybir.dt.int32
i16 = mybir.dt.int16

_NC = None


def _build():
    global _NC
    if _NC is not None:
        return _NC
    nc = bacc.Bacc("TRN2", target_bir_lowering=False, debug=False,
                   num_devices=N_CORES)

    # packed int16 input: [:, 0:B] = labels (replicated), [:, B:2B] = iota j
    pack16 = nc.declare_dram_parameter("pack16", [ACH, 2 * B], i16, isOutput=False)
    # tiny per-core inputs: [:, 0] = labels[anchor_p], [:, 1] = anchor id
    tiny16 = nc.declare_dram_parameter("tiny16", [ACH, 2], i16, isOutput=False)
    tinyf = nc.declare_dram_parameter("tinyf", [ACH, 2], f32, isOutput=False)

    anchor_out = nc.declare_dram_parameter("anchor_out", [ACH, PER, NNEG], i32, isOutput=True)
    pos_out = nc.declare_dram_parameter("pos_out", [ACH, PER, NNEG], i32, isOutput=True)
    neg_out = nc.declare_dram_parameter("neg_out", [ACH, PER, NNEG], i32, isOutput=True)

    op = mybir.AluOpType
    with tile.TileContext(nc) as tc:
        with tc.tile_pool(name="p", bufs=1) as pool:
            t_pack = pool.tile([ACH, 2 * B], i16)
            t_tiny16 = pool.tile([ACH, 2], i16)
            t_tinyf = pool.tile([ACH, 2], f32)
            t_ones = pool.tile([ACH, B], i16)
            t_eq = pool.tile([ACH, B], i16)
            t_rank = pool.tile([ACH, B], i16)
            t_tmpb = pool.tile([ACH, B], i16)   # j - rank
            t_x = pool.tile([ACH, B], i16)      # eq * j
            t_w = pool.tile([ACH, B], i16)      # eq*1024 - eq*j
            t_idx = pool.tile([ACH, B], i16)
            t_scat = pool.tile([ACH, B], i16)
            t_anc32 = pool.tile([ACH, NNEG], i32)
            t_uf = pool.tile([ACH, S], f32)
            t_cm = pool.tile([ACH, PER], f32)
            t_dq = pool.tile([ACH, PER], f32)
            t_dq2 = pool.tile([ACH, PER], f32)
            t_ppr = pool.tile([ACH, PER], f32)
            t_pos16 = pool.tile([ACH, PER, NNEG], i16)

            lab16 = t_pack[:, 0:B]
            iota16 = t_pack[:, B:2 * B]

            # inputs: tiny first (anchor path depends only on them)
            nc.sync.dma_start(t_tiny16[:, :], tiny16[:, :])
            nc.sync.dma_start(t_tinyf[:, :], tinyf[:, :])
            nc.sync.dma_start(t_pack[:, :], pack16[:, :])

            nc.vector.memset(t_ones[:, :], 1)

            # anchor slab: every element of row p is the global anchor id;
            # int32 tile, fanned out x7 on the ACT HWDGE ring.
            nc.vector.tensor_scalar(t_anc32[:, :], t_ones[:, :NNEG],
                                    0.0, t_tinyf[:, 1:2], op.mult, op.add)
            nc.scalar.dma_start(
                anchor_out[:, :, :],
                t_anc32[:, :].unsqueeze(1).broadcast_to([ACH, PER, NNEG]))

            # eq[p, j] = labels[j] == labels[anchor_p]
            nc.vector.tensor_scalar(t_eq[:, :], lab16,
                                    t_tinyf[:, 0:1], None, op.is_equal)
            # rank[p, j] = inclusive running count of members
            nc.vector.tensor_tensor_scan(t_rank[:, :], t_ones[:, :], t_eq[:, :],
                                         0.0, op.mult, op.add)
            # idx = (j - rank) + eq*(1024 - j): bijection on [0,1024)
            nc.vector.scalar_tensor_tensor(t_tmpb[:, :], t_rank[:, :], -1.0,
                                           iota16, op.mult, op.add)
            nc.vector.tensor_tensor(t_x[:, :], t_eq[:, :], iota16, op.mult)
            nc.vector.scalar_tensor_tensor(t_w[:, :], t_eq[:, :], 1024.0,
                                           t_x[:, :], op.mult, op.subtract)
            nc.vector.tensor_tensor(t_idx[:, :], t_tmpb[:, :], t_w[:, :], op.add)

            nc.gpsimd.local_scatter(t_scat[:, :], iota16, t_idx[:, :],
                                    channels=ACH, num_elems=B, num_idxs=B)

            # negatives: slots 0..1015; SWDGE DMA casts int16->int32, x7 fan-out
            nc.gpsimd.dma_start(
                neg_out[:, :, :],
                t_scat[:, :NNEG].unsqueeze(1).broadcast_to([ACH, PER, NNEG]))

            # members u_k = scat[1016+k] = q_{7-k} (descending).
            # ppRev[s] = u[s+1] if u[s+1] < anchor else u[s]; pp_t = ppRev[6-t].
            nc.vector.tensor_copy(t_uf[:, :], t_scat[:, NNEG:B])
            nc.vector.tensor_scalar(t_cm[:, :], t_uf[:, 1:S],
                                    t_tinyf[:, 1:2], None, op.is_lt)
            nc.vector.tensor_tensor(t_dq[:, :], t_uf[:, 1:S], t_uf[:, 0:PER], op.subtract)
            nc.vector.tensor_tensor(t_dq2[:, :], t_cm[:, :], t_dq[:, :], op.mult)
            nc.vector.tensor_tensor(t_ppr[:, :], t_uf[:, 0:PER], t_dq2[:, :], op.add)
            for t in range(PER):
                nc.vector.tensor_scalar(t_pos16[:, t, :], t_ones[:, :NNEG],
                                        0.0, t_ppr[:, PER - 1 - t:PER - t], op.mult, op.add)
            # contiguous SWDGE DMA with int16->int32 cast
            nc.gpsimd.dma_start(pos_out[:, :, :], t_pos16[:, :, :])
    nc.compile()
    _NC = nc
    return nc


def _in_maps(labels):
    lab = np.asarray(labels).astype(np.int16)
    pack = np.empty((ACH, 2 * B), dtype=np.int16)
    pack[:, 0:B] = lab[None, :]
    pack[:, B:2 * B] = np.arange(B, dtype=np.int16)[None, :]
    maps = []
    for d in range(N_CORES):
        sl = slice(d * ACH, (d + 1) * ACH)
        t16 = np.empty((ACH, 2), dtype=np.int16)
        t16[:, 0] = lab[sl]
        t16[:, 1] = np.arange(d * ACH, (d + 1) * ACH, dtype=np.int16)
        maps.append({"pack16": pack, "tiny16": t16,
                     "tinyf": t16.astype(np.float32)})
    return maps


def _gather(results):
    anchor = np.concatenate([results[d]["anchor_out"].reshape(-1)
                             for d in range(N_CORES)]).astype(np.int32, copy=False)
    pos = np.concatenate([results[d]["pos_out"].reshape(-1)
                          for d in range(N_CORES)]).astype(np.int32, copy=False)
    neg = np.concatenate([results[d]["neg_out"].reshape(-1)
                          for d in range(N_CORES)]).astype(np.int32, copy=False)
    return anchor, pos, neg


def run(labels, trace=False):
    nc = _build()
    res = run_bass_kernel_spmd(nc, _in_maps(labels),
                               core_ids=list(range(N_CORES)), trace=trace)
    return _gather(res.results), res


def kernel(embeddings=None, labels=None, **_):
    (anchor, pos, neg), _res = run(labels, trace=False)
    return anchor, pos, neg
